# revision 1
# baseline (speedup 1.0000x reference)
"""Trainium2 Bass kernel for CrossAttnMem (q-augmented cross attention with
a shared K/V memory bank, InstanceNorm'd scores, softmax over the bank axis).

Sharding: 8 query batches over 8 cores; each core holds its query slice plus
the full emb_u (replicated) and computes its output slice locally (no
collectives).

The K/V bank is rank-64 (a projection of the 64-channel emb_u), so every
bank-sized contraction is reassociated through the embedding dimension:

    M^T[b]  = emb_l^T @ emb_u[b]            [64, 64]   (K = n)
    G[b]    = (M^T[b])^T @ Wq^T             [64, CH]   (== emb_u[b]^T @ q)
    sT[b]   = Wk @ G[b]                     [CH, CH]   (scores, d-major)
    stats:    sum(s)  = sum_b <M^T[b], wq_rowsum x wk_colsum>
              sum(s^2)= sum_b <K2, M^T[b]^T Q2 M^T[b]>,  Q2 = Wq^T Wq,
                                                         K2 = Wk^T Wk
              (exact InstanceNorm stats via 64x64 trace identities)
    eT[b]   = exp(scale * sT[b] + bias)     (sT recomputed; exp straight from
                                             PSUM; no max needed: |s_n|<~8)
    H'[b]   = [Wv | 1]^T @ eT[b]            [65, CH]   (row 64 = Z_b)
    U(ns)  += H[b]^T-slices @ emb_u[b]^T    [CH, N]    (K = e, per n-half)
    Z       = sum_b H'[b][64]               -> transpose -> 1/Z per c
    out     = (U / Z)^T @ Wo^T

This cuts the bank-sized matmuls (N x CH x CH scores, CH x B*CH x N context)
down to rank-64 chains. All matmuls take fp16 inputs (full PE rate) with
fp32 PSUM accumulation; the stats path runs in fp32. Measured end-to-end
relative error vs the fp32 reference: ~6e-4.
"""

import os
import sys
from contextlib import ExitStack

import numpy as np

try:
    import concourse.bass as bass
except ImportError:  # pragma: no cover
    sys.path.insert(0, "/opt/trn_rl_repo")
    import concourse.bass as bass

import concourse.tile as tile
from concourse import bacc, mybir
from concourse.bass_utils import run_bass_kernel_spmd
from concourse.masks import make_identity

P = 128
N = 1024            # sequence length
E = 64              # embedding channels
CH = 512            # attention channels (num_heads * emb_ch)
B = 8               # kv batches (== upper half of emb batch)
NCORES = 8
NCH = N // P        # 8
CCH = CH // P       # 4
DCH = (B * CH) // P  # 32 d-chunks of the bank axis
EPS = 1e-5
NELEM = float(CH * B * CH)   # elements in one instance-norm plane

F32 = mybir.dt.float32
F16 = mybir.dt.float16
Exp = mybir.ActivationFunctionType.Exp
Sqrt = mybir.ActivationFunctionType.Sqrt
Identity = mybir.ActivationFunctionType.Identity
Mult = mybir.AluOpType.mult
Add = mybir.AluOpType.add
AxX = mybir.AxisListType.X


def build_kernel() -> bass.Bass:
    nc = bacc.Bacc("TRN2", debug=False, num_devices=NCORES)

    emb_l = nc.dram_tensor("emb_l", [N, E], F32, kind="ExternalInput")
    emb_u = nc.dram_tensor("emb_u", [B, N, E], F32, kind="ExternalInput")
    Wq = nc.dram_tensor("Wq", [CH, E], F32, kind="ExternalInput")
    Wk = nc.dram_tensor("Wk", [CH, E], F32, kind="ExternalInput")
    Wv = nc.dram_tensor("Wv", [CH, E], F32, kind="ExternalInput")
    Wo = nc.dram_tensor("Wo", [E, CH], F32, kind="ExternalInput")
    out = nc.dram_tensor("out", [N, E], F32, kind="ExternalOutput")

    with tile.TileContext(nc) as tc:
        _body(tc, emb_l, emb_u, Wq, Wk, Wv, Wo, out)
    nc.compile()
    return nc


def _body(tc, emb_l, emb_u, Wq, Wk, Wv, Wo, out):
    nc = tc.nc

    with ExitStack() as ctx:
        const = ctx.enter_context(tc.tile_pool(name="const", bufs=1))
        wpool = ctx.enter_context(tc.tile_pool(name="wpool", bufs=1))
        big = ctx.enter_context(tc.tile_pool(name="big", bufs=1))
        stream = ctx.enter_context(tc.tile_pool(name="stream", bufs=3))
        small = ctx.enter_context(tc.tile_pool(name="small", bufs=1))
        psum = ctx.enter_context(tc.tile_pool(name="psum", bufs=1, space="PSUM"))

        # PSUM budget (8 banks x 2KB/partition):
        #   tag "u": 2 x [128,2,512] f32 = 4 banks (stats 64x64s in pass A;
        #            the two U-accumulator pairs afterwards)
        #   tag "v": 1 x [128,2,512] f32 = 2 banks (G in pass A, sT recompute)
        #   tag "o": 1 x [65,512] f32   = 1 bank  (M^T, H', out projection)
        #   tag "z": 1 x <=2KB          = 1 bank  (transposes, stats, 1/Z)

        ident = const.tile([P, P], F32)
        make_identity(nc, ident)
        ident16 = const.tile([P, P], F16)
        nc.vector.tensor_copy(ident16[:], ident[:])

        # ---- weights ----
        # Wq^T, Wk^T: [E, CH] fp16 via PE transpose; keep fp16 naturals too
        wT = {}
        w16 = {}
        for wname, W in (("q", Wq), ("k", Wk)):
            w_nat = stream.tile([P, CCH, E], F32, tag="w_nat", bufs=2,
                                name=f"w_nat_{wname}")
            nc.sync.dma_start(w_nat[:], W.rearrange("(o p) e -> p o e", p=P))
            ps_wt = psum.tile([E, CH], F32, tag="z", bufs=1, name=f"ps_wt_{wname}")
            for o in range(CCH):
                nc.tensor.transpose(ps_wt[:, o * P:(o + 1) * P], w_nat[:, o, :],
                                    ident)
            wt = wpool.tile([E, CH], F16, name=f"wT_{wname}")
            nc.scalar.copy(wt[:], ps_wt[:])
            wT[wname] = wt
            wn16 = wpool.tile([P, CCH, E], F16, name=f"w16_{wname}")
            nc.vector.tensor_copy(wn16[:], w_nat[:])
            w16[wname] = wn16

        # Wv stays natural, augmented with a ones column (row 64 of H' = Z_b)
        w_natv = stream.tile([P, CCH, E], F32, tag="w_nat", bufs=2, name="w_natv")
        nc.sync.dma_start(w_natv[:], Wv.rearrange("(o p) e -> p o e", p=P))
        w_aug = wpool.tile([P, CCH, E + 1], F16, name="w_aug")
        nc.vector.tensor_copy(w_aug[:, :, 0:E], w_natv[:])
        nc.vector.memset(w_aug[:, :, E:E + 1], 1.0)

        # Wo^T: [E, CH] -> [CH, E] as [P, CCH, E] fp16
        wo_nat = stream.tile([E, CH], F32, tag="wo_nat", bufs=1, name="wo_nat")
        nc.sync.dma_start(wo_nat[:], Wo[:, :])
        ps_wo = psum.tile([P, CCH, E], F32, tag="z", bufs=1, name="ps_wo")
        for o in range(CCH):
            nc.tensor.transpose(ps_wo[:, o, :], wo_nat[:, o * P:(o + 1) * P],
                                ident[:E, :E])
        woT = wpool.tile([P, CCH, E], F16, name="woT")
        nc.scalar.copy(woT[:], ps_wo[:])

        # ---- stats constants: Q2 = Wq^T Wq, K2 = Wk^T Wk, W2 = outer sums --
        q2_sb = wpool.tile([E, E], F32, name="q2_sb")
        k2_sb = wpool.tile([E, E], F32, name="k2_sb")
        for wname, dst in (("q", q2_sb), ("k", k2_sb)):
            ps_w2m = psum.tile([E, E], F32, tag="u", bufs=2, name="ps_w2m")
            for o in range(CCH):
                nc.tensor.matmul(ps_w2m[:], w16[wname][:, o, :],
                                 w16[wname][:, o, :],
                                 start=(o == 0), stop=(o == CCH - 1))
            nc.vector.tensor_copy(dst[:], ps_w2m[:])
        # row sums of Wq^T / Wk^T over the CH axis
        wsum = small.tile([E, 2], F32, name="wsum")
        nc.vector.reduce_sum(wsum[:, 0:1], wT["q"][:], axis=AxX)
        nc.vector.reduce_sum(wsum[:, 1:2], wT["k"][:], axis=AxX)
        ps_wr = psum.tile([1, 2 * E], F32, tag="z", bufs=1, name="ps_wr")
        nc.tensor.transpose(ps_wr[:, 0:E], wsum[:, 0:1], ident[:E, :E])
        nc.tensor.transpose(ps_wr[:, E:2 * E], wsum[:, 1:2], ident[:E, :E])
        wr_sb = small.tile([1, 2 * E], F32, name="wr_sb")
        nc.vector.tensor_copy(wr_sb[:], ps_wr[:])
        ps_w2 = psum.tile([E, E], F32, tag="z", bufs=1, name="ps_w2")
        nc.tensor.matmul(ps_w2[:], wr_sb[:, 0:E], wr_sb[:, E:2 * E],
                         start=True, stop=True)
        w2_sb = wpool.tile([E, E], F32, name="w2_sb")
        nc.vector.tensor_copy(w2_sb[:], ps_w2[:])

        # ---- emb_l (natural, fp16) ----
        el_nat = stream.tile([P, NCH, E], F32, tag="emb_nat", bufs=2, name="el_nat")
        nc.sync.dma_start(el_nat[:], emb_l.rearrange("(nc p) e -> p nc e", p=P))
        el16 = wpool.tile([P, NCH, E], F16, name="el16")
        nc.vector.tensor_copy(el16[:], el_nat[:])

        # ---- pass A: per kv batch -> M^T, G, stats partials, emb_u^T ----
        euT = wpool.tile([E, B, N], F16, name="euT")
        G_sb = big.tile([E, B, CH], F16, name="G_sb")
        part_s = big.tile([E, B, 2], F32, name="part_s")
        for b in range(B):
            eu_nat = stream.tile([P, NCH, E], F32, tag="emb_nat", bufs=2,
                                 name="eu_nat")
            nc.sync.dma_start(eu_nat[:],
                              emb_u[b].rearrange("(nc p) e -> p nc e", p=P))
            eu16 = stream.tile([P, NCH, E], F16, tag="eu16", bufs=3, name="eu16")
            nc.vector.tensor_copy(eu16[:], eu_nat[:])

            # M^T[b] = emb_l^T @ emb_u[b]   [64, 64]
            ps_m = psum.tile([E, E], F32, tag="o", bufs=1, name="ps_m")
            for nch in range(NCH):
                nc.tensor.matmul(ps_m[:], el16[:, nch, :], eu16[:, nch, :],
                                 start=(nch == 0), stop=(nch == NCH - 1))
            m16 = stream.tile([E, E], F16, tag="m16", bufs=2, name="m16")
            nc.scalar.copy(m16[:], ps_m[:])
            m32 = stream.tile([E, E], F32, tag="m32", bufs=2, name="m32")
            nc.vector.tensor_copy(m32[:], ps_m[:])

            # G[b] = (M^T)^T @ Wq^T   [64, CH]
            ps_gg = psum.tile([E, CH], F32, tag="v", bufs=2, name="ps_gg")
            nc.tensor.matmul(ps_gg[:], m16[:], wT["q"][:], start=True, stop=True)
            nc.scalar.copy(G_sb[:, b, :], ps_gg[:])

            # stats partials: sum(s) via <M^T, W2>; sum(s^2) via <K2, P3>
            scr = stream.tile([E, E], F32, tag="scr", bufs=2, name="scr")
            nc.vector.tensor_mul(scr[:], m32[:], w2_sb[:])
            nc.vector.reduce_sum(part_s[:, b, 0:1], scr[:], axis=AxX)
            ps_p1 = psum.tile([E, E], F32, tag="u", bufs=2, name="ps_p1")
            nc.tensor.matmul(ps_p1[:], q2_sb[:], m32[:], start=True, stop=True)
            p1_sb = stream.tile([E, E], F32, tag="p1_sb", bufs=2, name="p1_sb")
            nc.scalar.copy(p1_sb[:], ps_p1[:])
            ps_p3 = psum.tile([E, E], F32, tag="u", bufs=2, name="ps_p3")
            nc.tensor.matmul(ps_p3[:], m32[:], p1_sb[:], start=True, stop=True)
            scr2 = stream.tile([E, E], F32, tag="scr2", bufs=2, name="scr2")
            nc.vector.tensor_mul(scr2[:], k2_sb[:], ps_p3[:])
            nc.vector.reduce_sum(part_s[:, b, 1:2], scr2[:], axis=AxX)

            # emb_u[b]^T (fp16), for the U contraction later
            for h in range(2):
                ps_et = psum.tile([E, 512], F16, tag="z", bufs=1, name="ps_eut")
                for j in range(4):
                    nch = h * 4 + j
                    nc.tensor.transpose(ps_et[:, j * P:(j + 1) * P],
                                        eu16[:, nch, :], ident16)
                if h == 0:
                    nc.vector.tensor_copy(euT[:, b, 0:512], ps_et[:])
                else:
                    nc.scalar.copy(euT[:, b, 512:1024], ps_et[:])

        # ---- global instance-norm stats ----
        ones_f = const.tile([P, 1], F32)
        nc.vector.memset(ones_f, 1.0)
        ones_row = const.tile([1, P], F32)
        nc.vector.memset(ones_row, 1.0)
        psums2 = small.tile([E, 2], F32, name="psums2")
        nc.vector.reduce_sum(psums2[:, 0:1], part_s[:, :, 0], axis=AxX)
        nc.vector.reduce_sum(psums2[:, 1:2], part_s[:, :, 1], axis=AxX)
        ps_g = psum.tile([1, 2], F32, tag="z", bufs=1, name="ps_g")
        nc.tensor.matmul(ps_g[:], ones_f[:E, :], psums2[:], start=True, stop=True)
        gm = small.tile([1, 2], F32, name="gm")
        nc.vector.tensor_scalar_mul(gm[:], ps_g[:], 1.0 / NELEM)  # [mu, E[s^2]]
        var = small.tile([1, 1], F32, name="var")
        nc.vector.tensor_mul(var[:], gm[:, 0:1], gm[:, 0:1])
        nc.vector.tensor_sub(var[:], gm[:, 1:2], var[:])
        sc = small.tile([1, 2], F32, name="sc")
        eps_t = small.tile([1, 1], F32, name="eps_t")
        nc.vector.memset(eps_t[:], EPS)
        nc.scalar.activation(sc[:, 0:1], var[:], Sqrt, bias=eps_t[:], scale=1.0)
        nc.vector.reciprocal(sc[:, 0:1], sc[:, 0:1])
        nc.vector.tensor_scalar(sc[:, 1:2], gm[:, 0:1], sc[:, 0:1], -1.0,
                                Mult, Mult)
        ps_bc = psum.tile([P, 2], F32, tag="z", bufs=1, name="ps_bc")
        nc.tensor.matmul(ps_bc[:], ones_row[:], sc[:], start=True, stop=True)
        sb_b = small.tile([P, 2], F32, name="sb_b")
        nc.vector.tensor_copy(sb_b[:], ps_bc[:])

        # ---- pass 0a: recompute sT -> exp -> H' (+Z row); U(ns=0, cc 0-1) ----
        H_sb = big.tile([E + 1, B, CH], F16, name="H_sb")
        ctx_bf = big.tile([P, CCH, N], F16, name="ctx_bf")
        out_sb = big.tile([P, NCH, E], F32, name="out_sb")
        # Z = sum_d eT[d, :] accumulates as a [1, CH] row (ones stationary)
        ones_16 = const.tile([P, 1], F16)
        nc.vector.memset(ones_16, 1.0)
        ps_z1 = psum.tile([1, CH], F32, tag="z", bufs=1, name="ps_z1")
        ups_g0 = [psum.tile([P, 512], F32, tag="u", bufs=2, name=f"ups_g0_{i}")
                  for i in range(2)]
        for b in range(B):
            eT_b = stream.tile([P, CCH, CH], F16, tag="eT_b", bufs=3, name="eT_b")
            for hp in range(2):
                ps_sb = psum.tile([P, 2, CH], F32, tag="v", bufs=2, name="ps_sb")
                for j in range(2):
                    cp = hp * 2 + j
                    nc.tensor.matmul(ps_sb[:, j, :],
                                     wT["k"][:, cp * P:(cp + 1) * P],
                                     G_sb[:, b, :], start=True, stop=True)
                nc.scalar.activation(eT_b[:, hp * 2:hp * 2 + 2, :], ps_sb[:],
                                     Exp, bias=sb_b[:, 1:2], scale=sb_b[:, 0:1])

            # H'[b] = [Wv | 1]^T @ eT[b]   [65, CH]; row 64 = Z_b
            ps_h = psum.tile([E + 1, CH], F32, tag="o", bufs=1, name="ps_h")
            for cp in range(CCH):
                nc.tensor.matmul(ps_h[:], w_aug[:, cp, :], eT_b[:, cp, :],
                                 start=(cp == 0), stop=(cp == CCH - 1))
            nc.vector.tensor_copy(H_sb[:, b, :], ps_h[:])
            for cp in range(CCH):
                nc.tensor.matmul(ps_z1[:], ones_16[:], eT_b[:, cp, :],
                                 start=(b == 0 and cp == 0),
                                 stop=(b == B - 1 and cp == CCH - 1))

            for cc in range(2):
                nc.tensor.matmul(ups_g0[cc][:],
                                 H_sb[0:E, b, cc * P:(cc + 1) * P],
                                 euT[:, b, 0:512],
                                 start=(b == 0), stop=(b == B - 1))

        # scatter Z row to c-partitions via K=1 matmuls, then invert
        z1_sb = small.tile([1, CH], F32, name="z1_sb")
        nc.vector.tensor_copy(z1_sb[:], ps_z1[:])
        ps_zt = psum.tile([P, CCH], F32, tag="z", bufs=1, name="ps_zt")
        for cc in range(CCH):
            nc.tensor.matmul(ps_zt[:, cc:cc + 1],
                             z1_sb[:, cc * P:(cc + 1) * P],
                             ident[0:1, 0:1],
                             start=(cc == 0), stop=(cc == CCH - 1))
        zr = small.tile([P, CCH], F32, name="zr")
        nc.vector.reciprocal(zr[:], ps_zt[:])

        def u_streak(ccs, ns):
            ups = [psum.tile([P, 512], F32, tag="u", bufs=2,
                             name=f"ups_{ns}_{cc}") for cc in ccs]
            for b in range(B):
                for i, cc in enumerate(ccs):
                    nc.tensor.matmul(ups[i][:],
                                     H_sb[0:E, b, cc * P:(cc + 1) * P],
                                     euT[:, b, ns * 512:(ns + 1) * 512],
                                     start=(b == 0), stop=(b == B - 1))
            return ups

        def ctx_div(ups, ccs, ns):
            for i, cc in enumerate(ccs):
                nc.vector.tensor_scalar_mul(
                    ctx_bf[:, cc, ns * 512:(ns + 1) * 512],
                    ups[i][:], zr[:, cc:cc + 1])

        def out_proj(ns):
            for j in range(4):
                nch = ns * 4 + j
                ps_o = psum.tile([P, E], F32, tag="v", bufs=2, name="ps_o")
                for cc in range(CCH):
                    nc.tensor.matmul(ps_o[:],
                                     ctx_bf[:, cc, nch * P:(nch + 1) * P],
                                     woT[:, cc, :],
                                     start=(cc == 0), stop=(cc == CCH - 1))
                if j % 2 == 0:
                    nc.scalar.copy(out_sb[:, nch, :], ps_o[:])
                else:
                    nc.vector.tensor_copy(out_sb[:, nch, :], ps_o[:])

        # ---- remaining U accumulations are pure matmul streaks ----
        ctx_div(ups_g0, (0, 1), 0)
        ups_g1 = u_streak((2, 3), 0)
        ctx_div(ups_g1, (2, 3), 0)
        out_proj(0)
        ups1_g0 = u_streak((0, 1), 1)
        ctx_div(ups1_g0, (0, 1), 1)
        ups1_g1 = u_streak((2, 3), 1)
        ctx_div(ups1_g1, (2, 3), 1)
        out_proj(1)

        nc.sync.dma_start(out.rearrange("(nc p) e -> p nc e", p=P), out_sb[:])


_NC_CACHE = None


def _get_nc():
    global _NC_CACHE
    if _NC_CACHE is None:
        _NC_CACHE = build_kernel()
    return _NC_CACHE


def kernel(**inputs) -> np.ndarray:
    emb = np.ascontiguousarray(np.asarray(inputs["emb"], dtype=np.float32))
    Wq = np.ascontiguousarray(np.asarray(inputs["Wq"], dtype=np.float32))
    Wk = np.ascontiguousarray(np.asarray(inputs["Wk"], dtype=np.float32))
    Wv = np.ascontiguousarray(np.asarray(inputs["Wv"], dtype=np.float32))
    Wo = np.ascontiguousarray(np.asarray(inputs["Wo"], dtype=np.float32))

    emb_u = np.ascontiguousarray(emb[:B])      # replicated K/V source
    in_maps = []
    for core in range(NCORES):
        in_maps.append({
            "emb_l": np.ascontiguousarray(emb[B + core]),
            "emb_u": emb_u,
            "Wq": Wq, "Wk": Wk, "Wv": Wv, "Wo": Wo,
        })

    nc = _get_nc()
    res = run_bass_kernel_spmd(nc, in_maps, core_ids=list(range(NCORES)))
    out = np.stack([res.results[c]["out"] for c in range(NCORES)], axis=0)
    return out.astype(np.float32)


if __name__ == "__main__":
    nc = build_kernel()
    print("built ok")



# revision 6
# speedup vs baseline: 1.2842x; 1.2842x over previous
"""Trainium2 Bass kernel for CrossAttnMem (q-augmented cross attention with
a shared K/V memory bank, InstanceNorm'd scores, softmax over the bank axis).

Sharding: 8 query batches over 8 cores; each core holds its query slice plus
the full emb_u (replicated) and computes its output slice locally (no
collectives).

The K/V bank is rank-64 (a projection of the 64-channel emb_u), so every
bank-sized contraction is reassociated through the embedding dimension:

    M^T[b]  = emb_l^T @ emb_u[b]                  [64, 64]   (K = n)
    G[b]    = (M^T[b])^T @ Wq^T                   [64, CH]
    sT[b]   = Wk @ G[b]                           [CH, CH]   (scores, d-major)
    stats:    sum(s)  = sum_b <M^T[b], wq_colsum x wk_colsum>
              sum(s^2)= sum_b <K2, M^T[b]^T Q2 M^T[b]>
    eT[b]   = exp(scale*sT[b] + bias - 4)         (exp from PSUM; global -4
                                                   shift keeps fp16 in range)
    Ht[b]   = eT[b]^T @ [Wv | 1]                  [CH, 65]  (col 64 = Z_b[c])
    Z[c]    = sum_b Ht[b][:, 64];  Wo' = Wo^T / Z [CH, 64]
    R[b]    = Ht[b]^T(e'-rows) ... R2 pair-stacked [128, 64]
    out     = sum_pairs euT2_pair^T-chunks @ R2   [N, 64]

i.e. the softmax denominator, the context contraction and the output
projection all collapse into per-b 64x64 "R" matrices; no [CH, N] context
tensor is ever materialized. All big matmuls run fp16 with fp32 PSUM.
"""

import sys
from contextlib import ExitStack

import numpy as np

try:
    import concourse.bass as bass
except ImportError:  # pragma: no cover
    sys.path.insert(0, "/opt/trn_rl_repo")
    import concourse.bass as bass

import concourse.tile as tile
from concourse import bacc, mybir
from concourse.bass_utils import run_bass_kernel_spmd
from concourse.masks import make_identity

P = 128
N = 1024            # sequence length
E = 64              # embedding channels
CH = 512            # attention channels
B = 8               # kv batches
NCORES = 8
NPAIR = 4           # kv-batch pairs
J = 8               # n-positions per partition (n = 8p + j)
JC = 4              # c-positions per partition for weights (c = 4p + j)
EPS = 1e-5
SHIFT = 4.0         # global softmax shift (exactly cancels; fp16 range aid)
NELEM = float(CH * B * CH)

F32 = mybir.dt.float32
F16 = mybir.dt.float16
Exp = mybir.ActivationFunctionType.Exp
Ln = mybir.ActivationFunctionType.Ln
Mult = mybir.AluOpType.mult
AxX = mybir.AxisListType.X


def build_kernel() -> bass.Bass:
    nc = bacc.Bacc("TRN2", debug=False, num_devices=NCORES)

    emb_l = nc.dram_tensor("emb_l", [N, E], F32, kind="ExternalInput")
    emb_u = nc.dram_tensor("emb_u", [B, N, E], F32, kind="ExternalInput")
    Wq = nc.dram_tensor("Wq", [CH, E], F32, kind="ExternalInput")
    Wk = nc.dram_tensor("Wk", [CH, E], F32, kind="ExternalInput")
    Wv = nc.dram_tensor("Wv", [CH, E], F32, kind="ExternalInput")
    Wo = nc.dram_tensor("Wo", [E, CH], F32, kind="ExternalInput")
    out = nc.dram_tensor("out", [N, E], F32, kind="ExternalOutput")

    with tile.TileContext(nc) as tc:
        _body(tc, emb_l, emb_u, Wq, Wk, Wv, Wo, out)
    nc.compile()
    return nc


def _body(tc, emb_l, emb_u, Wq, Wk, Wv, Wo, out):
    nc = tc.nc

    with ExitStack() as ctx:
        const = ctx.enter_context(tc.tile_pool(name="const", bufs=1))
        wpool = ctx.enter_context(tc.tile_pool(name="wpool", bufs=1))
        big = ctx.enter_context(tc.tile_pool(name="big", bufs=1))
        stream = ctx.enter_context(tc.tile_pool(name="stream", bufs=3))
        small = ctx.enter_context(tc.tile_pool(name="small", bufs=1))
        psum = ctx.enter_context(tc.tile_pool(name="psum", bufs=1, space="PSUM"))

        # PSUM budget (8 banks x 2KB/partition):
        #   tag z: 1 bank  (prep transients, stats p1/p3, tail R/R2)
        #   tag m: 1 bank  (M_cat; reused by the out accumulator)
        #   tag g: 2 banks (G per b; reused by Ht per b)
        #   tag v: 4 banks (euT2 transposes in pass A; sT 2x[128,2,512] after)

        # ---- input DMAs (DMA engines serialize: order = priority) ----
        el_nat = stream.tile([P, J, E], F32, tag="el_nat", bufs=1, name="el_nat")
        nc.sync.dma_start(el_nat[:], emb_l.rearrange("(p j) e -> p j e", p=P))
        w_nat = {}
        for wname, W in (("q", Wq), ("k", Wk)):
            wn = stream.tile([P, JC, E], F32, tag=f"wn_{wname}", bufs=1,
                             name=f"wn_{wname}")
            nc.sync.dma_start(wn[:], W.rearrange("(p j) e -> p j e", p=P))
            w_nat[wname] = wn
        eu_re = emb_u.rearrange("(b2 h) (p j) e -> p b2 h j e", b2=NPAIR, h=2,
                                p=P, j=J)
        eu_nat = big.tile([P, NPAIR, 2, J, E], F32, name="eu_nat")
        nc.sync.dma_start(eu_nat[:, 0], eu_re[:, 0])
        wv_nat = stream.tile([P, JC, E], F32, tag="wn_v", bufs=1, name="wn_v")
        nc.sync.dma_start(wv_nat[:], Wv.rearrange("(p j) e -> p j e", p=P))
        wo_nat = stream.tile([E, CH], F32, tag="wo_nat", bufs=1, name="wo_nat")
        nc.sync.dma_start(wo_nat[:], Wo[:, :])
        for pr in range(1, NPAIR):
            nc.sync.dma_start(eu_nat[:, pr], eu_re[:, pr])

        ident = const.tile([P, P], F32)
        make_identity(nc, ident)
        ident16 = const.tile([P, P], F16)
        nc.vector.tensor_copy(ident16[:], ident[:])
        ones_f = const.tile([E, 1], F32)
        nc.vector.memset(ones_f, 1.0)
        ones16 = const.tile([P, 1], F16)
        nc.vector.memset(ones16, 1.0)
        ones_row = const.tile([1, P], F32)
        nc.vector.memset(ones_row, 1.0)

        # ---- weight prep ----
        # fp16 naturals for Q2/K2 + colsum matmuls
        w16 = {}
        for wname in ("q", "k"):
            wn16 = wpool.tile([P, JC, E], F16, name=f"w16_{wname}")
            nc.vector.tensor_copy(wn16[:], w_nat[wname][:])
            w16[wname] = wn16
        # Wq^T / Wk^T: [E, JC, P] fp16, c = 4p + j
        wT = {}
        for wname in ("q", "k"):
            ps_wt = psum.tile([E, JC, P], F32, tag="z", bufs=1,
                              name=f"ps_wt_{wname}")
            for j in range(JC):
                nc.tensor.transpose(ps_wt[:, j, :], w_nat[wname][:, j, :], ident)
            wt = wpool.tile([E, JC, P], F16, name=f"wT_{wname}")
            nc.vector.tensor_copy(wt[:], ps_wt[:])
            wT[wname] = wt
        # w_aug = [Wv | 1]: [P, JC, E+1] fp16, d = 4p + j
        w_aug = wpool.tile([P, JC, E + 1], F16, name="w_aug")
        nc.vector.tensor_copy(w_aug[:, :, 0:E], wv_nat[:])
        nc.vector.memset(w_aug[:, :, E:E + 1], 1.0)
        # Wo^T: [P, JC, E] fp16, c = 4p + j (strided transpose input)
        ps_wo = psum.tile([P, JC, E], F32, tag="z", bufs=1, name="ps_wo")
        for j in range(JC):
            nc.tensor.transpose(ps_wo[:, j, :], wo_nat[:, j:CH:JC],
                                ident[:E, :E])
        woT = wpool.tile([P, JC, E], F16, name="woT")
        nc.vector.tensor_copy(woT[:], ps_wo[:])

        # stats constants: Q2 = Wq^T Wq, K2 = Wk^T Wk, colsums, W2 outer
        q2k2_ps = psum.tile([E, 2, E], F32, tag="z", bufs=1, name="q2k2_ps")
        for i, wname in enumerate(("q", "k")):
            for j in range(JC):
                nc.tensor.matmul(q2k2_ps[:, i, :], w16[wname][:, j, :],
                                 w16[wname][:, j, :],
                                 start=(j == 0), stop=(j == JC - 1))
        q2k2_16 = wpool.tile([E, 2, E], F16, name="q2k2_16")
        nc.vector.tensor_copy(q2k2_16[:], q2k2_ps[:])
        k2_sb = wpool.tile([E, E], F32, name="k2_sb")
        nc.vector.tensor_copy(k2_sb[:], q2k2_ps[:, 1, :])
        ps_ws = psum.tile([1, 2, E], F32, tag="z", bufs=1, name="ps_ws")
        for i, wname in enumerate(("q", "k")):
            for j in range(JC):
                nc.tensor.matmul(ps_ws[:, i, :], ones16[:, :],
                                 w16[wname][:, j, :],
                                 start=(j == 0), stop=(j == JC - 1))
        ws_sb = small.tile([1, 2, E], F16, name="ws_sb")
        nc.vector.tensor_copy(ws_sb[:], ps_ws[:])
        ps_w2 = psum.tile([E, E], F32, tag="z", bufs=1, name="ps_w2")
        nc.tensor.matmul(ps_w2[:], ws_sb[:, 0, :], ws_sb[:, 1, :],
                         start=True, stop=True)
        w2_sb = wpool.tile([E, E], F32, name="w2_sb")
        nc.vector.tensor_copy(w2_sb[:], ps_w2[:])

        el16 = wpool.tile([P, J, E], F16, name="el16")
        nc.vector.tensor_copy(el16[:], el_nat[:])

        # ---- pass A: per kv pair -> M^T, G, stats partials, euT2 ----
        eu16 = big.tile([P, NPAIR, J, P], F16, name="eu16")
        euT2 = big.tile([P, NPAIR, J, P], F16, name="euT2")
        m16 = big.tile([E, NPAIR, 2, E], F16, name="m16")
        G_sb = big.tile([E, B, JC, P], F16, name="G_sb")
        part_s = big.tile([E, B, 2], F32, name="part_s")
        ps_m = psum.tile([E, NPAIR, P], F32, tag="m", bufs=1, name="ps_m")
        for pr in range(NPAIR):
            # fp32 -> fp16 on ACT (idle during pass A; DVE is busy)
            for h in range(2):
                nc.scalar.copy(eu16[:, pr, :, h * E:(h + 1) * E],
                               eu_nat[:, pr, h, :, :])
            # M^T pair block: [e_l, (h e_u)]
            for j in range(J):
                nc.tensor.matmul(ps_m[:, pr, :], el16[:, j, :],
                                 eu16[:, pr, j, :],
                                 start=(j == 0), stop=(j == J - 1))
            nc.vector.tensor_copy(m16[:, pr, :, :], ps_m[:, pr, :])
            for h in range(2):
                b = 2 * pr + h
                # G[b] = (M^T)^T @ Wq^T -> [e_u, c]
                ps_g = psum.tile([E, CH], F32, tag="g", bufs=2, name="ps_g")
                nc.tensor.matmul(ps_g[:], m16[:, pr, h, :], wT["q"][:],
                                 start=True, stop=True)
                nc.vector.tensor_copy(G_sb[:, b, :, :], ps_g[:])
                # stats partials
                ps_p1 = psum.tile([E, E], F32, tag="z", bufs=1, name="ps_p1")
                nc.tensor.matmul(ps_p1[:], q2k2_16[:, 0, :], m16[:, pr, h, :],
                                 start=True, stop=True)
                p1_16 = stream.tile([E, E], F16, tag="p1_16", bufs=2,
                                    name="p1_16")
                nc.vector.tensor_copy(p1_16[:], ps_p1[:])
                ps_p3 = psum.tile([E, E], F32, tag="z", bufs=1, name="ps_p3")
                nc.tensor.matmul(ps_p3[:], m16[:, pr, h, :], p1_16[:],
                                 start=True, stop=True)
                scr = stream.tile([E, 2, E], F32, tag="scr", bufs=2, name="scr")
                nc.vector.tensor_mul(scr[:, 0, :],
                                     ps_m[:, pr, h * E:(h + 1) * E], w2_sb[:])
                nc.vector.tensor_mul(scr[:, 1, :], ps_p3[:], k2_sb[:])
                nc.vector.reduce_sum(part_s[:, b, 0:1], scr[:, 0, :], axis=AxX)
                nc.vector.reduce_sum(part_s[:, b, 1:2], scr[:, 1, :], axis=AxX)
            # euT2: transpose the side-by-side pair -> [(h e), p] stacked
            ps_t = psum.tile([P, J, P], F16, tag="v", bufs=2, name="ps_t")
            for j in range(J):
                nc.tensor.transpose(ps_t[:, j, :], eu16[:, pr, j, :], ident16)
            nc.vector.tensor_copy(euT2[:, pr, :, :], ps_t[:])

        # ---- global instance-norm stats -> exp scale/bias ----
        psums2 = small.tile([E, 2], F32, name="psums2")
        nc.vector.reduce_sum(psums2[:, 0:1], part_s[:, :, 0], axis=AxX)
        nc.vector.reduce_sum(psums2[:, 1:2], part_s[:, :, 1], axis=AxX)
        ps_gs = psum.tile([1, 2], F32, tag="z", bufs=1, name="ps_gs")
        nc.tensor.matmul(ps_gs[:], ones_f[:], psums2[:], start=True, stop=True)
        gm = small.tile([1, 2], F32, name="gm")
        nc.vector.tensor_scalar_mul(gm[:], ps_gs[:], 1.0 / NELEM)
        var = small.tile([1, 1], F32, name="var")
        nc.vector.tensor_mul(var[:], gm[:, 0:1], gm[:, 0:1])
        nc.vector.tensor_sub(var[:], gm[:, 1:2], var[:])
        lnv = small.tile([1, 1], F32, name="lnv")
        eps_t = small.tile([1, 1], F32, name="eps_t")
        nc.vector.memset(eps_t[:], EPS)
        nc.scalar.activation(lnv[:], var[:], Ln, bias=eps_t[:])
        sc = small.tile([1, 2], F32, name="sc")
        nc.scalar.activation(sc[:, 0:1], lnv[:], Exp, scale=-0.5)
        nc.vector.tensor_mul(sc[:, 1:2], gm[:, 0:1], sc[:, 0:1])
        nc.vector.tensor_scalar(sc[:, 1:2], sc[:, 1:2], -1.0, -SHIFT,
                                Mult, mybir.AluOpType.add)
        ps_bc = psum.tile([P, 2], F32, tag="z", bufs=1, name="ps_bc")
        nc.tensor.matmul(ps_bc[:], ones_row[:], sc[:], start=True, stop=True)
        sb_b = small.tile([P, 2], F32, name="sb_b")
        nc.vector.tensor_copy(sb_b[:], ps_bc[:])

        # ---- exp phase: sT -> exp -> Ht, pipelined over b ----
        Ht_sb = big.tile([P, B, JC, E + 1], F16, name="Ht_sb")
        eT = {}

        def issue_sT_exp(b):
            eT_b = stream.tile([P, JC, CH], F16, tag="eT", bufs=2, name="eT")
            eT[b] = eT_b
            for hp in range(2):
                ps_s = psum.tile([P, 2, CH], F32, tag="v", bufs=2, name="ps_s")
                for i in range(2):
                    dj = hp * 2 + i
                    nc.tensor.matmul(ps_s[:, i, :], wT["k"][:, dj, :],
                                     G_sb[:, b, :, :], start=True, stop=True)
                nc.scalar.activation(eT_b[:, hp * 2:hp * 2 + 2, :], ps_s[:],
                                     Exp, bias=sb_b[:, 1:2], scale=sb_b[:, 0:1])

        def issue_Ht(b):
            ps_h = psum.tile([P, JC, E + 1], F32, tag="g", bufs=2, name="ps_h")
            for cc in range(JC):
                for dj in range(JC):
                    nc.tensor.matmul(ps_h[:, cc, :],
                                     eT[b][:, dj, cc * P:(cc + 1) * P],
                                     w_aug[:, dj, :],
                                     start=(dj == 0), stop=(dj == JC - 1))
            nc.vector.tensor_copy(Ht_sb[:, b, :, :], ps_h[:])
            del eT[b]

        issue_sT_exp(0)
        for b in range(1, B):
            issue_sT_exp(b)
            issue_Ht(b - 1)
        issue_Ht(B - 1)

        # ---- tail: Z -> Wo' -> R2 pairs -> out ----
        zsum = small.tile([P, JC], F32, name="zsum")
        for jc in range(JC):
            nc.vector.reduce_sum(zsum[:, jc:jc + 1], Ht_sb[:, :, jc, E:E + 1],
                                 axis=mybir.AxisListType.XY)
        zr = small.tile([P, JC], F32, name="zr")
        nc.vector.reciprocal(zr[:], zsum[:])
        woS = wpool.tile([P, JC, E], F16, name="woS")
        for jc in range(JC):
            nc.vector.tensor_scalar_mul(woS[:, jc, :], woT[:, jc, :],
                                        zr[:, jc:jc + 1])

        out_ps = psum.tile([P, J, E], F32, tag="m", bufs=1, name="out_ps")
        r2_sb = big.tile([P, NPAIR, E], F16, name="r2_sb")
        for pr in range(NPAIR):
            rT_ps = psum.tile([E, 2, E], F32, tag="z", bufs=1, name="rT_ps")
            for h in range(2):
                b = 2 * pr + h
                for jc in range(JC):
                    nc.tensor.matmul(rT_ps[:, h, :], woS[:, jc, :],
                                     Ht_sb[:, b, jc, 0:E],
                                     start=(jc == 0), stop=(jc == JC - 1))
            rT16 = stream.tile([E, 2 * E], F16, tag="rT16", bufs=2, name="rT16")
            for h in range(2):
                nc.vector.tensor_copy(rT16[:, h * E:(h + 1) * E],
                                      rT_ps[:, h, :])
            r2_ps = psum.tile([P, E], F16, tag="z", bufs=1, name="r2_ps")
            nc.tensor.transpose(r2_ps[:], rT16[:], ident16[:E, :E])
            nc.vector.tensor_copy(r2_sb[:, pr, :], r2_ps[:])

        out_sb = big.tile([P, J, E], F32, name="out_sb")
        out_re = out.rearrange("(p j) e -> p j e", p=P)
        for half in range(2):
            for j in range(J // 2 * half, J // 2 * (half + 1)):
                for pr in range(NPAIR):
                    nc.tensor.matmul(out_ps[:, j, :], euT2[:, pr, j, :],
                                     r2_sb[:, pr, :],
                                     start=(pr == 0), stop=(pr == NPAIR - 1))
            h0 = J // 2 * half
            h1 = J // 2 * (half + 1)
            nc.vector.tensor_copy(out_sb[:, h0:h1, :], out_ps[:, h0:h1, :])
            nc.sync.dma_start(out_re[:, h0:h1, :], out_sb[:, h0:h1, :])


_NC_CACHE = None


def _get_nc():
    global _NC_CACHE
    if _NC_CACHE is None:
        _NC_CACHE = build_kernel()
    return _NC_CACHE


def kernel(**inputs) -> np.ndarray:
    emb = np.ascontiguousarray(np.asarray(inputs["emb"], dtype=np.float32))
    Wq = np.ascontiguousarray(np.asarray(inputs["Wq"], dtype=np.float32))
    Wk = np.ascontiguousarray(np.asarray(inputs["Wk"], dtype=np.float32))
    Wv = np.ascontiguousarray(np.asarray(inputs["Wv"], dtype=np.float32))
    Wo = np.ascontiguousarray(np.asarray(inputs["Wo"], dtype=np.float32))

    emb_u = np.ascontiguousarray(emb[:B])      # replicated K/V source
    in_maps = []
    for core in range(NCORES):
        in_maps.append({
            "emb_l": np.ascontiguousarray(emb[B + core]),
            "emb_u": emb_u,
            "Wq": Wq, "Wk": Wk, "Wv": Wv, "Wo": Wo,
        })

    nc = _get_nc()
    res = run_bass_kernel_spmd(nc, in_maps, core_ids=list(range(NCORES)))
    out = np.stack([res.results[c]["out"] for c in range(NCORES)], axis=0)
    return out.astype(np.float32)


if __name__ == "__main__":
    nc = build_kernel()
    print("built ok")


# revision 24
# speedup vs baseline: 1.4804x; 1.1528x over previous
"""Trainium2 Bass kernel for CrossAttnMem (q-augmented cross attention with
a shared K/V memory bank, InstanceNorm'd scores, softmax over the bank axis).

Sharding: 8 query batches over 8 cores; each core holds its query slice plus
the full emb_u (replicated) and computes its output slice locally (no
collectives).

The K/V bank is rank-64 (a projection of the 64-channel emb_u), so every
bank-sized contraction is reassociated through the embedding dimension:

    M^T[b]  = emb_l^T @ emb_u[b]                  [64, 64]   (K = n)
    G[b]    = (M^T[b])^T @ Wq^T                   [64, CH]
    sT[b]   = Wk @ G[b]                           [CH, CH]   (scores, d-major)
    stats:    sum(s)  = sum_b <M^T[b], wq_colsum x wk_colsum>
              sum(s^2)= sum_b <K2, M^T[b]^T Q2 M^T[b]>
    eT[b]   = exp(scale*sT[b] + bias - 4)         (exp from PSUM; global -4
                                                   shift keeps fp16 in range)
    Ht[b]   = eT[b]^T @ [Wv | 1]                  [CH, 65]  (col 64 = Z_b[c])
    Z[c]    = sum_b Ht[b][:, 64];  Wo' = Wo^T / Z [CH, 64]
    R2[pair]= Ht-pair^T(c) @ Wo'                  [128, 64]  (e' pair-stacked)
    out     = sum_pairs euT2_pair^T-chunks @ R2   [N, 64]

i.e. the softmax denominator, the context contraction and the output
projection all collapse into per-pair rank-128 "R2" matrices; no [CH, N]
context tensor is ever materialized. Engine split: PE does all contractions,
ACT does ln/exp + a few fp32->fp16 conversions, Pool converts emb_u, DVE
does PSUM drains and the norm-stats reductions.
"""

import sys
from contextlib import ExitStack

import numpy as np

try:
    import concourse.bass as bass
except ImportError:  # pragma: no cover
    sys.path.insert(0, "/opt/trn_rl_repo")
    import concourse.bass as bass

import concourse.tile as tile
from concourse import bacc, mybir
from concourse.bass_utils import run_bass_kernel_spmd
from concourse.masks import make_identity

P = 128
N = 1024            # sequence length
E = 64              # embedding channels
CH = 512            # attention channels
B = 8               # kv batches
NCORES = 8
NPAIR = 4           # kv-batch pairs
J = 8               # n-positions per partition (n = 8p + j)
JC = 4              # c-positions per partition for weights (c = 4p + j)
EPS = 1e-5
SHIFT = 4.0         # global softmax shift (exactly cancels; fp16 range aid)
NELEM = float(CH * B * CH)

F32 = mybir.dt.float32
F16 = mybir.dt.float16
Exp = mybir.ActivationFunctionType.Exp
Ln = mybir.ActivationFunctionType.Ln
Mult = mybir.AluOpType.mult
Add = mybir.AluOpType.add
Sub = mybir.AluOpType.subtract
AxX = mybir.AxisListType.X
AxXYZ = mybir.AxisListType.XYZ


def build_kernel() -> bass.Bass:
    nc = bacc.Bacc("TRN2", debug=False, num_devices=NCORES)

    emb_l = nc.dram_tensor("emb_l", [N, E], F32, kind="ExternalInput")
    emb_u = nc.dram_tensor("emb_u", [B, N, E], F32, kind="ExternalInput")
    Wq = nc.dram_tensor("Wq", [CH, E], F32, kind="ExternalInput")
    Wk = nc.dram_tensor("Wk", [CH, E], F32, kind="ExternalInput")
    Wv = nc.dram_tensor("Wv", [CH, E], F32, kind="ExternalInput")
    Wo = nc.dram_tensor("Wo", [E, CH], F32, kind="ExternalInput")
    out = nc.dram_tensor("out", [N, E], F32, kind="ExternalOutput")

    with tile.TileContext(nc) as tc:
        _body(tc, emb_l, emb_u, Wq, Wk, Wv, Wo, out)
    nc.compile()
    return nc


def _body(tc, emb_l, emb_u, Wq, Wk, Wv, Wo, out):
    nc = tc.nc

    with ExitStack() as ctx:
        const = ctx.enter_context(tc.tile_pool(name="const", bufs=1))
        wpool = ctx.enter_context(tc.tile_pool(name="wpool", bufs=1))
        big = ctx.enter_context(tc.tile_pool(name="big", bufs=1))
        stream = ctx.enter_context(tc.tile_pool(name="stream", bufs=3))
        small = ctx.enter_context(tc.tile_pool(name="small", bufs=1))
        psum = ctx.enter_context(tc.tile_pool(name="psum", bufs=1, space="PSUM"))

        # PSUM budget (8 banks x 2KB/partition):
        #   tag z: 2 banks (prep transients, stats p1/p3, global broadcasts,
        #                   Ht half-tiles, tail R2 -- rotation-ordered)
        #   tag m: 2 banks (per-pair M^T; then euT2; then out accum)
        #   tag g: 4 banks (G [64,512] in pass A; sT [128,2,512] x2 in exp)

        # ---- input DMAs (DMA engines serialize: order = priority) ----
        eu_re = emb_u.rearrange("(b2 h) (p j) e -> p b2 h j e", b2=NPAIR, h=2,
                                p=P, j=J)
        eu_nat = big.tile([P, NPAIR, 2, J, E], F32, name="eu_nat")
        el_nat = stream.tile([P, J, E], F32, tag="el_nat", bufs=1, name="el_nat")
        nc.sync.dma_start(el_nat[:], emb_l.rearrange("(p j) e -> p j e", p=P))
        nc.sync.dma_start(eu_nat[:, 0], eu_re[:, 0])
        w_nat = {}
        for wname, W in (("q", Wq), ("k", Wk)):
            wn = stream.tile([P, JC, E], F32, tag=f"wn_{wname}", bufs=1,
                             name=f"wn_{wname}")
            nc.sync.dma_start(wn[:], W.rearrange("(p j) e -> p j e", p=P))
            w_nat[wname] = wn
        nc.sync.dma_start(eu_nat[:, 1], eu_re[:, 1])
        nc.sync.dma_start(eu_nat[:, 2], eu_re[:, 2])
        nc.sync.dma_start(eu_nat[:, 3], eu_re[:, 3])
        wv_nat = stream.tile([P, JC, E], F32, tag="wn_v", bufs=1, name="wn_v")
        nc.sync.dma_start(wv_nat[:], Wv.rearrange("(p j) e -> p j e", p=P))
        wo_nat = stream.tile([E, CH], F32, tag="wo_nat", bufs=1, name="wo_nat")
        nc.sync.dma_start(wo_nat[:], Wo[:, :])

        ident = const.tile([P, P], F32)
        make_identity(nc, ident)
        ident16 = const.tile([P, P], F16)
        nc.vector.tensor_copy(ident16[:], ident[:])
        ones16 = const.tile([P, 1], F16)
        nc.vector.memset(ones16, 1.0)
        ones_row = const.tile([1, P], F32)
        nc.vector.memset(ones_row, 1.0)
        neg_row = const.tile([1, P], F32)
        nc.vector.memset(neg_row, -1.0)
        zeros_p = const.tile([P, J, E], F32)
        nc.vector.memset(zeros_p, 0.0)
        eps_t = small.tile([1, 1], F32, name="eps_t")
        nc.vector.memset(eps_t[:], EPS)
        # dummy Ln: pulls the natural_log_exp table load into the DMA window
        dum = small.tile([1, 1], F32, name="dum")
        nc.scalar.activation(dum[:], eps_t[:], Ln)

        # ---- weight prep ----
        w16 = {}
        for wname in ("q", "k"):
            wn16 = wpool.tile([P, JC, E], F16, name=f"w16_{wname}")
            nc.vector.tensor_copy(wn16[:], w_nat[wname][:])
            w16[wname] = wn16
        wT = {}
        for wname in ("q", "k"):
            ps_wt = psum.tile([E, JC, P], F32, tag="z", bufs=2,
                              name=f"ps_wt_{wname}")
            for j in range(JC):
                nc.tensor.transpose(ps_wt[:, j, :], w_nat[wname][:, j, :], ident)
            wt = wpool.tile([E, JC, P], F16, name=f"wT_{wname}")
            nc.vector.tensor_copy(wt[:], ps_wt[:])
            wT[wname] = wt
        w_aug = wpool.tile([P, JC, E + 1], F16, name="w_aug")
        nc.scalar.copy(w_aug[:, :, 0:E], wv_nat[:])
        nc.vector.memset(w_aug[:, :, E:E + 1], 1.0)
        # stats constants: Q2 = Wq^T Wq, K2 = Wk^T Wk, colsums, W2 outer
        q2k2_ps = psum.tile([E, 2, E], F32, tag="z", bufs=2, name="q2k2_ps")
        for i, wname in enumerate(("q", "k")):
            for j in range(JC):
                nc.tensor.matmul(q2k2_ps[:, i, :], w16[wname][:, j, :],
                                 w16[wname][:, j, :],
                                 start=(j == 0), stop=(j == JC - 1))
        q2_16 = wpool.tile([E, E], F16, name="q2_16")
        nc.vector.tensor_copy(q2_16[:], q2k2_ps[:, 0, :])
        k2_sb = wpool.tile([E, E], F32, name="k2_sb")
        nc.vector.tensor_copy(k2_sb[:], q2k2_ps[:, 1, :])
        ps_ws = psum.tile([1, 2, E], F32, tag="z", bufs=2, name="ps_ws")
        for i, wname in enumerate(("q", "k")):
            for j in range(JC):
                nc.tensor.matmul(ps_ws[:, i, :], ones16[:, :],
                                 w16[wname][:, j, :],
                                 start=(j == 0), stop=(j == JC - 1))
        ws_sb = small.tile([1, 2, E], F16, name="ws_sb")
        nc.vector.tensor_copy(ws_sb[:], ps_ws[:])
        ps_w2 = psum.tile([E, E], F32, tag="z", bufs=2, name="ps_w2")
        nc.tensor.matmul(ps_w2[:], ws_sb[:, 0, :], ws_sb[:, 1, :],
                         start=True, stop=True)
        w2_16 = wpool.tile([E, E], F16, name="w2_16")
        nc.vector.tensor_copy(w2_16[:], ps_w2[:])

        el16 = wpool.tile([P, J, E], F16, name="el16")
        nc.vector.tensor_copy(el16[:], el_nat[:])

        # ---- pass A: per kv pair -> M^T, G, instance-norm stat partials ----
        # do-ahead structure: pair pr's M matmuls issue before pair pr-1's
        # stats chain so PE never stalls behind a cross-engine wait.
        eu16 = big.tile([P, NPAIR, J, P], F16, name="eu16")
        m16 = big.tile([E, NPAIR, 2, E], F16, name="m16")
        G_sb = big.tile([E, B, JC, P], F16, name="G_sb")
        part_s = big.tile([E, 2, B], F32, name="part_s")
        scr = big.tile([E, 2, E], F32, name="scr")

        def issue_m(pr):
            for h in range(2):
                dst = eu16[:, pr, :, h * E:(h + 1) * E]
                if pr == NPAIR - 1 and h == 1:
                    nc.vector.tensor_copy(dst, eu_nat[:, pr, h, :, :])
                else:
                    # gpsimd "copy": only TensorTensor has a Q7 kernel
                    nc.gpsimd.tensor_tensor(dst, eu_nat[:, pr, h, :, :],
                                            zeros_p[:], Add)
            ps_m = psum.tile([E, P], F32, tag="m", bufs=2, name="ps_m")
            for j in range(J):
                nc.tensor.matmul(ps_m[:], el16[:, j, :], eu16[:, pr, j, :],
                                 start=(j == 0), stop=(j == J - 1))
            nc.vector.tensor_copy(m16[:, pr, :, :], ps_m[:])

        def issue_stats(pr):
            ps_p1 = psum.tile([E, 2, E], F32, tag="z", bufs=2, name="ps_p1")
            for h in range(2):
                nc.tensor.matmul(ps_p1[:, h, :], q2_16[:], m16[:, pr, h, :],
                                 start=True, stop=True)
            p1_16 = stream.tile([E, 2, E], F16, tag="p1_16", bufs=2,
                                name="p1_16")
            nc.vector.tensor_copy(p1_16[:], ps_p1[:])
            for h in range(2):
                ps_g = psum.tile([E, CH], F32, tag="g", bufs=2, name="ps_g")
                nc.tensor.matmul(ps_g[:], m16[:, pr, h, :], wT["q"][:],
                                 start=True, stop=True)
                nc.vector.tensor_copy(G_sb[:, 2 * pr + h, :, :], ps_g[:])
            ps_p3 = psum.tile([E, 2, E], F32, tag="z", bufs=2, name="ps_p3")
            for h in range(2):
                b = 2 * pr + h
                nc.tensor.matmul(ps_p3[:, h, :], m16[:, pr, h, :],
                                 p1_16[:, h, :], start=True, stop=True)
                nc.vector.tensor_mul(scr[:, 0, :], m16[:, pr, h, :], w2_16[:])
                nc.vector.tensor_mul(scr[:, 1, :], ps_p3[:, h, :], k2_sb[:])
                nc.vector.reduce_sum(part_s[:, 0, b:b + 1], scr[:, 0, :],
                                     axis=AxX)
                nc.vector.reduce_sum(part_s[:, 1, b:b + 1], scr[:, 1, :],
                                     axis=AxX)

        issue_m(0)
        for pr in range(1, NPAIR):
            issue_m(pr)
            issue_stats(pr - 1)
        issue_stats(NPAIR - 1)

        # ---- global instance-norm stats -> exp scale/bias ----
        psums2 = small.tile([E, 2], F32, name="psums2")
        nc.vector.reduce_sum(psums2[:], part_s[:], axis=AxX)
        ps_gs = psum.tile([1, 2], F32, tag="z", bufs=2, name="ps_gs")
        onesN = const.tile([E, 1], F32)
        nc.vector.memset(onesN, 1.0 / NELEM)
        nc.tensor.matmul(ps_gs[:], onesN[:], psums2[:], start=True, stop=True)
        gsum = small.tile([1, 2], F32, name="gsum")
        nc.vector.tensor_copy(gsum[:], ps_gs[:])
        varm = small.tile([1, 1], F32, name="varm")
        nc.vector.tensor_mul(varm[:], gsum[:, 0:1], gsum[:, 0:1])
        var = small.tile([1, 1], F32, name="var")
        nc.vector.tensor_sub(var[:], gsum[:, 1:2], varm[:])
        lnv = small.tile([1, 1], F32, name="lnv")
        nc.scalar.activation(lnv[:], var[:], Ln, bias=eps_t[:])
        rsig = small.tile([1, 1], F32, name="rsig")
        nc.scalar.activation(rsig[:], lnv[:], Exp, scale=-0.5)
        t1 = small.tile([1, 1], F32, name="t1")
        nc.vector.tensor_mul(t1[:], gsum[:, 0:1], rsig[:])
        ps_bc = psum.tile([P, 2], F32, tag="z", bufs=2, name="ps_bc")
        nc.tensor.matmul(ps_bc[:, 0:1], ones_row[:], rsig[:],
                         start=True, stop=True)
        nc.tensor.matmul(ps_bc[:, 1:2], neg_row[:], t1[:],
                         start=True, stop=True)
        sb_scale = small.tile([P, 1], F32, name="sb_scale")
        nc.vector.tensor_copy(sb_scale[:], ps_bc[:, 0:1])
        sb_bias = small.tile([P, 1], F32, name="sb_bias")
        nc.vector.tensor_scalar_add(sb_bias[:], ps_bc[:, 1:2], -SHIFT)

        # Wo^T (off the stats critical path; needed only in the tail)
        ps_wo = psum.tile([P, JC, E], F32, tag="z", bufs=2, name="ps_wo")
        for j in range(JC):
            nc.tensor.transpose(ps_wo[:, j, :], wo_nat[:, j:CH:JC],
                                ident[:E, :E])
        woT = wpool.tile([P, JC, E], F16, name="woT")
        nc.vector.tensor_copy(woT[:], ps_wo[:])

        # ---- exp phase: sT -> exp -> Ht, pipelined over b; euT2 woven in ----
        # Ht_sb layout [c-part, jc, pair, h, e'] so pair-stacked R2 reads it
        # directly as a [c, 128] stationary operand.
        Ht_sb = big.tile([P, JC, NPAIR, 2 * E], F16, name="Ht_sb")
        zcol = big.tile([P, JC, NPAIR, 2], F16, name="zcol")
        euT2 = big.tile([P, NPAIR, J, P], F16, name="euT2")
        eT = {}

        def issue_sT_exp(b):
            eT_b = stream.tile([P, JC, CH], F16, tag="eT", bufs=2, name="eT")
            eT[b] = eT_b
            for hp in range(2):
                ps_s = psum.tile([P, 2, CH], F32, tag="g", bufs=2, name="ps_s")
                for i in range(2):
                    dj = hp * 2 + i
                    nc.tensor.matmul(ps_s[:, i, :], wT["k"][:, dj, :],
                                     G_sb[:, b, :, :], start=True, stop=True)
                nc.scalar.activation(eT_b[:, hp * 2:hp * 2 + 2, :], ps_s[:],
                                     Exp, bias=sb_bias[:], scale=sb_scale[:])

        def issue_Ht(b):
            pr, hh = b // 2, b % 2
            for ch in range(2):
                ps_h = psum.tile([P, 2, E + 1], F32, tag="z", bufs=2,
                                 name="ps_h")
                for i in range(2):
                    cc = 2 * ch + i
                    for dj in range(JC):
                        nc.tensor.matmul(ps_h[:, i, :],
                                         eT[b][:, dj, cc * P:(cc + 1) * P],
                                         w_aug[:, dj, :],
                                         start=(dj == 0), stop=(dj == JC - 1))
                nc.vector.tensor_copy(
                    Ht_sb[:, 2 * ch:2 * ch + 2, pr, hh * E:(hh + 1) * E],
                    ps_h[:, :, 0:E])
                nc.vector.tensor_copy(
                    zcol[:, 2 * ch:2 * ch + 2, pr, hh:hh + 1],
                    ps_h[:, :, E:E + 1])
            del eT[b]

        def issue_euT2(pr):
            ps_t = psum.tile([P, J, P], F16, tag="m", bufs=2, name="ps_t")
            for j in range(J):
                nc.tensor.transpose(ps_t[:, j, :], eu16[:, pr, j, :], ident16)
            nc.vector.tensor_copy(euT2[:, pr, :, :], ps_t[:])

        issue_sT_exp(0)
        for b in range(1, B):
            issue_sT_exp(b)
            issue_Ht(b - 1)
            if b % 2 == 0:
                issue_euT2(b // 2 - 1)
        issue_Ht(B - 1)
        issue_euT2(NPAIR - 1)

        # ---- tail: Z -> Wo' -> R2 pairs -> out ----
        zsum = small.tile([P, JC], F32, name="zsum")
        for jc in range(JC):
            nc.vector.reduce_sum(zsum[:, jc:jc + 1], zcol[:, jc, :, :],
                                 axis=mybir.AxisListType.XY)
        zr = small.tile([P, JC], F32, name="zr")
        nc.vector.reciprocal(zr[:], zsum[:])
        woS = wpool.tile([P, JC, E], F16, name="woS")
        for jc in range(JC):
            nc.vector.tensor_scalar_mul(woS[:, jc, :], woT[:, jc, :],
                                        zr[:, jc:jc + 1])

        r2_sb = big.tile([P, NPAIR, E], F16, name="r2_sb")
        for pr in range(NPAIR):
            r2_ps = psum.tile([P, E], F32, tag="z", bufs=2, name="r2_ps")
            for jc in range(JC):
                nc.tensor.matmul(r2_ps[:], Ht_sb[:, jc, pr, :],
                                 woS[:, jc, :],
                                 start=(jc == 0), stop=(jc == JC - 1))
            nc.vector.tensor_copy(r2_sb[:, pr, :], r2_ps[:])

        out_ps = psum.tile([P, J, E], F32, tag="m", bufs=2, name="out_ps")
        out_sb = big.tile([P, J, E], F32, name="out_sb")
        out_re = out.rearrange("(p j) e -> p j e", p=P)
        for j in range(J):
            for pr in range(NPAIR):
                nc.tensor.matmul(out_ps[:, j, :], euT2[:, pr, j, :],
                                 r2_sb[:, pr, :],
                                 start=(pr == 0), stop=(pr == NPAIR - 1))
        for half in range(2):
            h0 = J // 2 * half
            h1 = J // 2 * (half + 1)
            nc.scalar.copy(out_sb[:, h0:h1, :], out_ps[:, h0:h1, :])
            nc.sync.dma_start(out_re[:, h0:h1, :], out_sb[:, h0:h1, :])


_NC_CACHE = None


def _get_nc():
    global _NC_CACHE
    if _NC_CACHE is None:
        _NC_CACHE = build_kernel()
    return _NC_CACHE


def kernel(**inputs) -> np.ndarray:
    emb = np.ascontiguousarray(np.asarray(inputs["emb"], dtype=np.float32))
    Wq = np.ascontiguousarray(np.asarray(inputs["Wq"], dtype=np.float32))
    Wk = np.ascontiguousarray(np.asarray(inputs["Wk"], dtype=np.float32))
    Wv = np.ascontiguousarray(np.asarray(inputs["Wv"], dtype=np.float32))
    Wo = np.ascontiguousarray(np.asarray(inputs["Wo"], dtype=np.float32))

    emb_u = np.ascontiguousarray(emb[:B])      # replicated K/V source
    in_maps = []
    for core in range(NCORES):
        in_maps.append({
            "emb_l": np.ascontiguousarray(emb[B + core]),
            "emb_u": emb_u,
            "Wq": Wq, "Wk": Wk, "Wv": Wv, "Wo": Wo,
        })

    nc = _get_nc()
    res = run_bass_kernel_spmd(nc, in_maps, core_ids=list(range(NCORES)))
    out = np.stack([res.results[c]["out"] for c in range(NCORES)], axis=0)
    return out.astype(np.float32)


if __name__ == "__main__":
    nc = build_kernel()
    print("built ok")


# revision 26
# speedup vs baseline: 1.6868x; 1.1394x over previous
"""Trainium2 Bass kernel for CrossAttnMem (q-augmented cross attention with
a shared K/V memory bank, InstanceNorm'd scores, softmax over the bank axis).

Sharding: 8 query batches over 8 cores; each core holds its query slice plus
the full emb_u (replicated) and computes its output slice locally (no
collectives).

The K/V bank is rank-64 (a projection of the 64-channel emb_u), so every
bank-sized contraction is reassociated through the embedding dimension:

    M^T[b]  = emb_l^T @ emb_u[b]                  [64, 64]   (K = n)
    G[b]    = (M^T[b])^T @ Wq^T                   [64, CH]
    sT[b]   = Wk @ G[b]                           [CH, CH]   (scores, d-major)
    stats:    sum(s)  = sum_b <M^T[b], wq_colsum x wk_colsum>
              sum(s^2)= sum_b <K2, M^T[b]^T Q2 M^T[b]>
    eT[b]   = exp(scale*sT[b] + bias - 4)         (exp from PSUM; global -4
                                                   shift keeps fp16 in range)
    Ht[b]   = eT[b]^T @ [Wv | 1]                  [CH, 65]  (col 64 = Z_b[c])
    Z[c]    = sum_b Ht[b][:, 64];  Wo' = Wo^T / Z [CH, 64]
    R2[pair]= Ht-pair^T(c) @ Wo'                  [128, 64]  (e' pair-stacked)
    out     = sum_pairs euT2_pair^T-chunks @ R2   [N, 64]

i.e. the softmax denominator, the context contraction and the output
projection all collapse into per-pair rank-128 "R2" matrices; no [CH, N]
context tensor is ever materialized. Engine split: PE does all contractions,
ACT does ln/exp + a few fp32->fp16 conversions, Pool converts emb_u, DVE
does PSUM drains and the norm-stats reductions.
"""

import sys
from contextlib import ExitStack

import numpy as np

try:
    import concourse.bass as bass
except ImportError:  # pragma: no cover
    sys.path.insert(0, "/opt/trn_rl_repo")
    import concourse.bass as bass

import concourse.tile as tile
from concourse import bacc, mybir
from concourse.bass_utils import run_bass_kernel_spmd
from concourse.masks import make_identity

P = 128
N = 1024            # sequence length
E = 64              # embedding channels
CH = 512            # attention channels
B = 8               # kv batches
NCORES = 8
NPAIR = 4           # kv-batch pairs
J = 8               # n-positions per partition (n = 8p + j)
JC = 4              # c-positions per partition for weights (c = 4p + j)
EPS = 1e-5
SHIFT = 4.0         # global softmax shift (exactly cancels; fp16 range aid)
NELEM = float(CH * B * CH)

F32 = mybir.dt.float32
F16 = mybir.dt.float16
Exp = mybir.ActivationFunctionType.Exp
Ln = mybir.ActivationFunctionType.Ln
Mult = mybir.AluOpType.mult
Add = mybir.AluOpType.add
Sub = mybir.AluOpType.subtract
AxX = mybir.AxisListType.X
AxXYZ = mybir.AxisListType.XYZ


def build_kernel() -> bass.Bass:
    nc = bacc.Bacc("TRN2", debug=False, num_devices=NCORES)

    emb_l = nc.dram_tensor("emb_l", [N, E], F16, kind="ExternalInput")
    emb_u = nc.dram_tensor("emb_u", [B, N, E], F16, kind="ExternalInput")
    Wq = nc.dram_tensor("Wq", [CH, E], F32, kind="ExternalInput")
    Wk = nc.dram_tensor("Wk", [CH, E], F32, kind="ExternalInput")
    Wv = nc.dram_tensor("Wv", [CH, E], F32, kind="ExternalInput")
    Wo = nc.dram_tensor("Wo", [E, CH], F32, kind="ExternalInput")
    out = nc.dram_tensor("out", [N, E], F32, kind="ExternalOutput")

    with tile.TileContext(nc) as tc:
        _body(tc, emb_l, emb_u, Wq, Wk, Wv, Wo, out)
    nc.compile()
    return nc


def _body(tc, emb_l, emb_u, Wq, Wk, Wv, Wo, out):
    nc = tc.nc

    with ExitStack() as ctx:
        const = ctx.enter_context(tc.tile_pool(name="const", bufs=1))
        wpool = ctx.enter_context(tc.tile_pool(name="wpool", bufs=1))
        big = ctx.enter_context(tc.tile_pool(name="big", bufs=1))
        stream = ctx.enter_context(tc.tile_pool(name="stream", bufs=3))
        small = ctx.enter_context(tc.tile_pool(name="small", bufs=1))
        psum = ctx.enter_context(tc.tile_pool(name="psum", bufs=1, space="PSUM"))

        # PSUM budget (8 banks x 2KB/partition):
        #   tag z: 2 banks (prep transients, stats p1/p3, global broadcasts,
        #                   Ht half-tiles, tail R2 -- rotation-ordered)
        #   tag m: 2 banks (per-pair M^T; then euT2; then out accum)
        #   tag g: 4 banks (G [64,512] in pass A; sT [128,2,512] x2 in exp)

        # ---- input DMAs (DMA engines serialize: order = priority) ----
        # emb is cast to fp16 on the host: halves the dominant transfers and
        # removes the on-chip convert layer (compute consumed fp16 anyway)
        eu_re = emb_u.rearrange("(b2 h) (p j) e -> p b2 h j e", b2=NPAIR, h=2,
                                p=P, j=J)
        eu16 = big.tile([P, NPAIR, 2, J, E], F16, name="eu16")
        el16 = big.tile([P, J, E], F16, name="el16")
        nc.sync.dma_start(eu16[:, 0], eu_re[:, 0])
        nc.sync.dma_start(el16[:], emb_l.rearrange("(p j) e -> p j e", p=P))
        w_nat = {}
        for wname, W in (("q", Wq), ("k", Wk)):
            wn = stream.tile([P, JC, E], F32, tag=f"wn_{wname}", bufs=1,
                             name=f"wn_{wname}")
            nc.sync.dma_start(wn[:], W.rearrange("(p j) e -> p j e", p=P))
            w_nat[wname] = wn
        nc.sync.dma_start(eu16[:, 1], eu_re[:, 1])
        nc.sync.dma_start(eu16[:, 2], eu_re[:, 2])
        nc.sync.dma_start(eu16[:, 3], eu_re[:, 3])
        wv_nat = stream.tile([P, JC, E], F32, tag="wn_v", bufs=1, name="wn_v")
        nc.sync.dma_start(wv_nat[:], Wv.rearrange("(p j) e -> p j e", p=P))
        wo_nat = stream.tile([E, CH], F32, tag="wo_nat", bufs=1, name="wo_nat")
        nc.sync.dma_start(wo_nat[:], Wo[:, :])

        ident = const.tile([P, P], F32)
        make_identity(nc, ident)
        ident16 = const.tile([P, P], F16)
        nc.vector.tensor_copy(ident16[:], ident[:])
        ones16 = const.tile([P, 1], F16)
        nc.vector.memset(ones16, 1.0)
        ones_row = const.tile([1, P], F32)
        nc.vector.memset(ones_row, 1.0)
        neg_row = const.tile([1, P], F32)
        nc.vector.memset(neg_row, -1.0)
        eps_t = small.tile([1, 1], F32, name="eps_t")
        nc.vector.memset(eps_t[:], EPS)
        # dummy Ln: pulls the natural_log_exp table load into the DMA window
        dum = small.tile([1, 1], F32, name="dum")
        nc.scalar.activation(dum[:], eps_t[:], Ln)

        # ---- weight prep ----
        w16 = {}
        for wname in ("q", "k"):
            wn16 = wpool.tile([P, JC, E], F16, name=f"w16_{wname}")
            nc.vector.tensor_copy(wn16[:], w_nat[wname][:])
            w16[wname] = wn16
        wT = {}
        for wname in ("q", "k"):
            ps_wt = psum.tile([E, JC, P], F32, tag="z", bufs=2,
                              name=f"ps_wt_{wname}")
            for j in range(JC):
                nc.tensor.transpose(ps_wt[:, j, :], w_nat[wname][:, j, :], ident)
            wt = wpool.tile([E, JC, P], F16, name=f"wT_{wname}")
            nc.vector.tensor_copy(wt[:], ps_wt[:])
            wT[wname] = wt
        w_aug = wpool.tile([P, JC, E + 1], F16, name="w_aug")
        nc.vector.tensor_copy(w_aug[:, :, 0:E], wv_nat[:])
        nc.vector.memset(w_aug[:, :, E:E + 1], 1.0)
        # stats constants: Q2 = Wq^T Wq, K2 = Wk^T Wk, colsums, W2 outer
        q2k2_ps = psum.tile([E, 2, E], F32, tag="z", bufs=2, name="q2k2_ps")
        for i, wname in enumerate(("q", "k")):
            for j in range(JC):
                nc.tensor.matmul(q2k2_ps[:, i, :], w16[wname][:, j, :],
                                 w16[wname][:, j, :],
                                 start=(j == 0), stop=(j == JC - 1))
        q2_16 = wpool.tile([E, E], F16, name="q2_16")
        nc.vector.tensor_copy(q2_16[:], q2k2_ps[:, 0, :])
        k2_sb = wpool.tile([E, E], F32, name="k2_sb")
        nc.vector.tensor_copy(k2_sb[:], q2k2_ps[:, 1, :])
        ps_ws = psum.tile([1, 2, E], F32, tag="z", bufs=2, name="ps_ws")
        for i, wname in enumerate(("q", "k")):
            for j in range(JC):
                nc.tensor.matmul(ps_ws[:, i, :], ones16[:, :],
                                 w16[wname][:, j, :],
                                 start=(j == 0), stop=(j == JC - 1))
        ws_sb = small.tile([1, 2, E], F16, name="ws_sb")
        nc.vector.tensor_copy(ws_sb[:], ps_ws[:])
        ps_w2 = psum.tile([E, E], F32, tag="z", bufs=2, name="ps_w2")
        nc.tensor.matmul(ps_w2[:], ws_sb[:, 0, :], ws_sb[:, 1, :],
                         start=True, stop=True)
        w2_16 = wpool.tile([E, E], F16, name="w2_16")
        nc.vector.tensor_copy(w2_16[:], ps_w2[:])

        # ---- pass A: per kv pair -> M^T, G, instance-norm stat partials ----
        # do-ahead structure: pair pr's M matmuls issue before pair pr-1's
        # stats chain so PE never stalls behind a cross-engine wait.
        m16 = big.tile([E, NPAIR, 2, E], F16, name="m16")
        G_sb = big.tile([E, B, JC, P], F16, name="G_sb")
        part_s = big.tile([E, 2, B], F32, name="part_s")
        scr = big.tile([E, 2, E], F32, name="scr")

        def issue_m(pr):
            ps_m = psum.tile([E, P], F32, tag="m", bufs=2, name="ps_m")
            for j in range(J):
                nc.tensor.matmul(ps_m[:], el16[:, j, :], eu16[:, pr, :, j, :],
                                 start=(j == 0), stop=(j == J - 1))
            nc.vector.tensor_copy(m16[:, pr, :, :], ps_m[:])

        def issue_stats(pr):
            ps_p1 = psum.tile([E, 2, E], F32, tag="z", bufs=2, name="ps_p1")
            for h in range(2):
                nc.tensor.matmul(ps_p1[:, h, :], q2_16[:], m16[:, pr, h, :],
                                 start=True, stop=True)
            p1_16 = stream.tile([E, 2, E], F16, tag="p1_16", bufs=2,
                                name="p1_16")
            nc.vector.tensor_copy(p1_16[:], ps_p1[:])
            for h in range(2):
                ps_g = psum.tile([E, CH], F32, tag="g", bufs=2, name="ps_g")
                nc.tensor.matmul(ps_g[:], m16[:, pr, h, :], wT["q"][:],
                                 start=True, stop=True)
                nc.scalar.copy(G_sb[:, 2 * pr + h, :, :], ps_g[:])
            ps_p3 = psum.tile([E, 2, E], F32, tag="z", bufs=2, name="ps_p3")
            for h in range(2):
                b = 2 * pr + h
                nc.tensor.matmul(ps_p3[:, h, :], m16[:, pr, h, :],
                                 p1_16[:, h, :], start=True, stop=True)
                nc.vector.tensor_mul(scr[:, 0, :], m16[:, pr, h, :], w2_16[:])
                nc.vector.tensor_mul(scr[:, 1, :], ps_p3[:, h, :], k2_sb[:])
                nc.vector.reduce_sum(part_s[:, 0, b:b + 1], scr[:, 0, :],
                                     axis=AxX)
                nc.vector.reduce_sum(part_s[:, 1, b:b + 1], scr[:, 1, :],
                                     axis=AxX)

        issue_m(0)
        for pr in range(1, NPAIR):
            issue_m(pr)
            issue_stats(pr - 1)
        issue_stats(NPAIR - 1)

        # ---- global instance-norm stats -> exp scale/bias ----
        psums2 = small.tile([E, 2], F32, name="psums2")
        nc.vector.reduce_sum(psums2[:], part_s[:], axis=AxX)
        ps_gs = psum.tile([1, 2], F32, tag="z", bufs=2, name="ps_gs")
        onesN = const.tile([E, 1], F32)
        nc.vector.memset(onesN, 1.0 / NELEM)
        nc.tensor.matmul(ps_gs[:], onesN[:], psums2[:], start=True, stop=True)
        gsum = small.tile([1, 2], F32, name="gsum")
        nc.vector.tensor_copy(gsum[:], ps_gs[:])
        varm = small.tile([1, 1], F32, name="varm")
        nc.vector.tensor_mul(varm[:], gsum[:, 0:1], gsum[:, 0:1])
        var = small.tile([1, 1], F32, name="var")
        nc.vector.tensor_sub(var[:], gsum[:, 1:2], varm[:])
        lnv = small.tile([1, 1], F32, name="lnv")
        nc.scalar.activation(lnv[:], var[:], Ln, bias=eps_t[:])
        rsig = small.tile([1, 1], F32, name="rsig")
        nc.scalar.activation(rsig[:], lnv[:], Exp, scale=-0.5)
        t1 = small.tile([1, 1], F32, name="t1")
        nc.vector.tensor_mul(t1[:], gsum[:, 0:1], rsig[:])
        ps_bc = psum.tile([P, 2], F32, tag="z", bufs=2, name="ps_bc")
        nc.tensor.matmul(ps_bc[:, 0:1], ones_row[:], rsig[:],
                         start=True, stop=True)
        nc.tensor.matmul(ps_bc[:, 1:2], neg_row[:], t1[:],
                         start=True, stop=True)
        sb_scale = small.tile([P, 1], F32, name="sb_scale")
        nc.vector.tensor_copy(sb_scale[:], ps_bc[:, 0:1])
        sb_bias = small.tile([P, 1], F32, name="sb_bias")
        nc.vector.tensor_scalar_add(sb_bias[:], ps_bc[:, 1:2], -SHIFT)

        # Wo^T (off the stats critical path; needed only in the tail)
        ps_wo = psum.tile([P, JC, E], F32, tag="z", bufs=2, name="ps_wo")
        for j in range(JC):
            nc.tensor.transpose(ps_wo[:, j, :], wo_nat[:, j:CH:JC],
                                ident[:E, :E])
        woT = wpool.tile([P, JC, E], F16, name="woT")
        nc.vector.tensor_copy(woT[:], ps_wo[:])

        # ---- exp phase: sT -> exp -> Ht, pipelined over b; euT2 woven in ----
        # Ht_sb layout [c-part, jc, pair, h, e'] so pair-stacked R2 reads it
        # directly as a [c, 128] stationary operand.
        Ht_sb = big.tile([P, JC, NPAIR, 2 * E], F16, name="Ht_sb")
        zcol = big.tile([P, JC, NPAIR, 2], F16, name="zcol")
        euT2 = big.tile([P, NPAIR, J, P], F16, name="euT2")
        eT = {}

        def issue_sT_exp(b):
            eT_b = stream.tile([P, JC, CH], F16, tag="eT", bufs=2, name="eT")
            eT[b] = eT_b
            for hp in range(2):
                ps_s = psum.tile([P, 2, CH], F32, tag="g", bufs=2, name="ps_s")
                for i in range(2):
                    dj = hp * 2 + i
                    nc.tensor.matmul(ps_s[:, i, :], wT["k"][:, dj, :],
                                     G_sb[:, b, :, :], start=True, stop=True)
                nc.scalar.activation(eT_b[:, hp * 2:hp * 2 + 2, :], ps_s[:],
                                     Exp, bias=sb_bias[:], scale=sb_scale[:])

        def issue_Ht(b):
            pr, hh = b // 2, b % 2
            for ch in range(2):
                ps_h = psum.tile([P, 2, E + 1], F32, tag="z", bufs=2,
                                 name="ps_h")
                for i in range(2):
                    cc = 2 * ch + i
                    for dj in range(JC):
                        nc.tensor.matmul(ps_h[:, i, :],
                                         eT[b][:, dj, cc * P:(cc + 1) * P],
                                         w_aug[:, dj, :],
                                         start=(dj == 0), stop=(dj == JC - 1))
                nc.vector.tensor_copy(
                    Ht_sb[:, 2 * ch:2 * ch + 2, pr, hh * E:(hh + 1) * E],
                    ps_h[:, :, 0:E])
                nc.vector.tensor_copy(
                    zcol[:, 2 * ch:2 * ch + 2, pr, hh:hh + 1],
                    ps_h[:, :, E:E + 1])
            del eT[b]

        def issue_euT2(pr):
            ps_t = psum.tile([P, J, P], F16, tag="m", bufs=2, name="ps_t")
            for j in range(J):
                for h in range(2):
                    nc.tensor.transpose(ps_t[h * E:(h + 1) * E, j, :],
                                        eu16[:, pr, h, j, :], ident16)
            nc.vector.tensor_copy(euT2[:, pr, :, :], ps_t[:])

        issue_sT_exp(0)
        for b in range(1, B):
            issue_sT_exp(b)
            issue_Ht(b - 1)
            if b % 2 == 0:
                issue_euT2(b // 2 - 1)
        issue_Ht(B - 1)
        issue_euT2(NPAIR - 1)

        # ---- tail: Z -> Wo' -> R2 pairs -> out ----
        zsum = small.tile([P, JC], F32, name="zsum")
        for jc in range(JC):
            nc.vector.reduce_sum(zsum[:, jc:jc + 1], zcol[:, jc, :, :],
                                 axis=mybir.AxisListType.XY)
        zr = small.tile([P, JC], F32, name="zr")
        nc.vector.reciprocal(zr[:], zsum[:])
        woS = wpool.tile([P, JC, E], F16, name="woS")
        for jc in range(JC):
            nc.vector.tensor_scalar_mul(woS[:, jc, :], woT[:, jc, :],
                                        zr[:, jc:jc + 1])

        r2_sb = big.tile([P, NPAIR, E], F16, name="r2_sb")
        for pr in range(NPAIR):
            r2_ps = psum.tile([P, E], F32, tag="z", bufs=2, name="r2_ps")
            for jc in range(JC):
                nc.tensor.matmul(r2_ps[:], Ht_sb[:, jc, pr, :],
                                 woS[:, jc, :],
                                 start=(jc == 0), stop=(jc == JC - 1))
            nc.vector.tensor_copy(r2_sb[:, pr, :], r2_ps[:])

        out_ps = psum.tile([P, J, E], F32, tag="m", bufs=2, name="out_ps")
        out_sb = big.tile([P, J, E], F32, name="out_sb")
        out_re = out.rearrange("(p j) e -> p j e", p=P)
        for j in range(J):
            for pr in range(NPAIR):
                nc.tensor.matmul(out_ps[:, j, :], euT2[:, pr, j, :],
                                 r2_sb[:, pr, :],
                                 start=(pr == 0), stop=(pr == NPAIR - 1))
        for half in range(2):
            h0 = J // 2 * half
            h1 = J // 2 * (half + 1)
            nc.scalar.copy(out_sb[:, h0:h1, :], out_ps[:, h0:h1, :])
            nc.sync.dma_start(out_re[:, h0:h1, :], out_sb[:, h0:h1, :])


_NC_CACHE = None


def _get_nc():
    global _NC_CACHE
    if _NC_CACHE is None:
        _NC_CACHE = build_kernel()
    return _NC_CACHE


def kernel(**inputs) -> np.ndarray:
    emb = np.ascontiguousarray(np.asarray(inputs["emb"], dtype=np.float32)
                               .astype(np.float16))
    Wq = np.ascontiguousarray(np.asarray(inputs["Wq"], dtype=np.float32))
    Wk = np.ascontiguousarray(np.asarray(inputs["Wk"], dtype=np.float32))
    Wv = np.ascontiguousarray(np.asarray(inputs["Wv"], dtype=np.float32))
    Wo = np.ascontiguousarray(np.asarray(inputs["Wo"], dtype=np.float32))

    emb_u = np.ascontiguousarray(emb[:B])      # replicated K/V source
    in_maps = []
    for core in range(NCORES):
        in_maps.append({
            "emb_l": np.ascontiguousarray(emb[B + core]),
            "emb_u": emb_u,
            "Wq": Wq, "Wk": Wk, "Wv": Wv, "Wo": Wo,
        })

    nc = _get_nc()
    res = run_bass_kernel_spmd(nc, in_maps, core_ids=list(range(NCORES)))
    out = np.stack([res.results[c]["out"] for c in range(NCORES)], axis=0)
    return out.astype(np.float32)


if __name__ == "__main__":
    nc = build_kernel()
    print("built ok")
